# revision 29
# baseline (speedup 1.0000x reference)
"""Trainium2 distributed kernel for the ACSConv Chebyshev graph conv.

Math (reference): with Tx0 = tile(x, (8,1)) [16384,16],
    Tx_{k} = 2*Ls@Tx_{k-1} - Tx_{k-2}   (Tx1 = Ls@Tx0)
    out = sum_k proj(Tx_k, W_k) + bias,  proj mixes the 8 angle blocks.

Distribution (8 NeuronCores): Ls row-sharded into contiguous 2048-row
blocks (= angle blocks). Core i holds LsT_i = Ls[2048i:2048(i+1), :].T
(host pre-transposed, bf16) and computes its Tx block each iteration;
blocks are AllGather'd between iterations.

Own-block-first overlap: each core's LsT slots are host-rotated so
slots 0..15 contract against the core's OWN Tx block. Those slots are
fully local — stationary comes from the just-computed `znat` tile and
the Ls data is SBUF-resident — so the per-iteration AllGather (and the
stationary loads that follow it) overlap ~22us of DMA-free PE work at
each iteration boundary instead of stalling both PE and the Ls stream.
After the AllGather, the 8 blocks land in a doubled [15-block] SBUF
window (two parallel SWDGE DMAs), and one dynamic-offset DVE copy
extracts this rank's rotated window [rank+1, rank+8) as the stationary
tile for the 7 peer-block slot groups.

Per-core per-iteration matmul: Y.T[16, 2048] = sum_g TxTile_g.T @ LsT_g
with TxTile [128,16] stationary and LsT tiles [128, 512] moving. Slots
16..127 stream via 2 MiB DMAs (DMA-native host layout: each partition
reads DMA_KT*4KB contiguous) alternating between the Sync and Scalar
HWDGE rings. Recurrence/projection/accumulation stay in fp32; the Ls
stream and stationary Tx tiles are bf16 (sim rel-err ~6e-3, gate 2e-2).
"""

import numpy as np
import ml_dtypes

import concourse.bass as bass
import concourse.mybir as mybir
import concourse.tile as tile
from concourse import bacc
from concourse.bass_utils import run_bass_kernel_spmd
from concourse.masks import make_identity

NCORES = 8
N = 2048          # nodes
CIN = 16
COUT = 32
NANG = 8          # angles
K = 15            # Chebyshev order
NATOT = NANG * N  # 16384
RPC = NATOT // NCORES   # rows of Ls per core = 2048
G = NATOT // 128        # 128 contraction k-tiles
MCH = RPC // 512        # 4 output m-chunks of 512
TPG = N // 128          # 16 row-tiles per core block
W_AG = TPG * CIN        # 256: per-partition payload of the AG

BF16 = mybir.dt.bfloat16
F32 = mybir.dt.float32
NP_BF16 = ml_dtypes.bfloat16

RES_KT = TPG      # own-block slots 0..15 are SBUF-resident
STREAM_KT = G - RES_KT
DMA_KT = 2        # k-tiles per streaming DMA (1 MiB)
LS_BUFS = 11      # streaming prefetch depth (x DMA_KT tiles)

_NC_CACHE = {}


def _build():
    nc = bacc.Bacc("TRN2", target_bir_lowering=False, debug=False,
                   num_devices=NCORES, num_swdge_queues=2)

    lst_s = nc.dram_tensor("lst_s", [STREAM_KT // DMA_KT, 128, DMA_KT, RPC],
                           BF16, kind="ExternalInput")
    lst_res = nc.dram_tensor("lst_res", [128, RES_KT, RPC], BF16,
                             kind="ExternalInput")
    xb = nc.dram_tensor("xb", [N, CIN], BF16, kind="ExternalInput")
    xt = nc.dram_tensor("xt", [CIN, N], F32, kind="ExternalInput")
    w = nc.dram_tensor("w", [K, CIN, COUT], F32, kind="ExternalInput")
    out = nc.dram_tensor("out", [COUT, RPC], F32, kind="ExternalOutput")

    with tile.TileContext(nc) as tc:
        with (
            tc.tile_pool(name="ls", bufs=LS_BUFS) as ls_pool,
            tc.tile_pool(name="tx", bufs=2) as tx_pool,
            tc.tile_pool(name="zt", bufs=3) as zt_pool,
            tc.tile_pool(name="znat", bufs=2) as znat_pool,
            tc.tile_pool(name="small", bufs=1) as small,
            tc.tile_pool(name="yps", bufs=4, space="PSUM") as yps,
            tc.tile_pool(name="pps", bufs=2, space="PSUM") as pps,
            tc.tile_pool(name="tps", bufs=2, space="PSUM") as tps,
            tc.tile_pool(name="dram", bufs=1, space="DRAM") as dram,
        ):
            # ---- preamble ----
            ident = small.tile([CIN, CIN], F32)
            make_identity(nc, ident[:])

            xb_sb = small.tile([128, TPG, CIN], BF16)
            nc.gpsimd.dma_start(xb_sb[:], xb.ap().rearrange("(t p) c -> p t c", p=128))

            w_sb = small.tile([CIN, K, COUT], F32)
            nc.gpsimd.dma_start(w_sb[:], w.ap().rearrange("k p c -> p k c"))

            ls_res = small.tile([128, RES_KT, RPC], BF16)
            half = RES_KT // 2
            nc.sync.dma_start(ls_res[:, :half, :], lst_res[:, :half, :])
            nc.scalar.dma_start(ls_res[:, half:, :], lst_res[:, half:, :])

            # rank+1: dynamic offset of the rotated stationary window
            # (register on DVE, which performs the rotated copy)
            rot_off = nc.vector.partition_id() + 1

            # dummy AllGather, same shape as the real ones: absorbs the
            # first-collective warmup cost under iteration-1 compute
            warm_in = dram.tile([128, W_AG], BF16, name="warm_in", tag="agin")
            warm_out = dram.tile([NCORES * 128, W_AG], BF16, name="warm_out",
                                 tag="agout", addr_space="Shared")
            nc.gpsimd.dma_start(warm_in[:], xb_sb[:])
            nc.gpsimd.collective_compute(
                "AllGather", mybir.AluOpType.bypass,
                replica_groups=[list(range(NCORES))],
                ins=[warm_in.opt()], outs=[warm_out.opt()])

            # zts[0] = x.T in fp32 (Tx0 block transposed), lives in the zt pool
            xt_sb = zt_pool.tile([CIN, N], F32, name="xt_sb", tag="zt")
            nc.gpsimd.dma_start(xt_sb[:], xt[:])

            acc = small.tile([COUT, RPC], F32)
            for j in range(MCH):
                pj = pps.tile([COUT, 512], F32, name="pj", tag="proj")
                nc.tensor.matmul(pj[:], w_sb[:, 0, :], xt_sb[:, j * 512:(j + 1) * 512],
                                 start=True, stop=True)
                nc.vector.tensor_copy(acc[:, j * 512:(j + 1) * 512], pj[:])

            # ---- Chebyshev iterations k = 1..14 ----
            zts = {0: xt_sb}
            znat_prev = None
            for k in range(1, K):
                # Y.T = (Ls_i @ Tx_{k-1}).T accumulated over the 128 slots
                ys = [yps.tile([CIN, 512], F32, name=f"y{j}", tag="y")
                      for j in range(MCH)]
                if k == 1:
                    def lhs(g):
                        return xb_sb[:, g % TPG, :]
                else:
                    zn, txr = znat_prev, tx_rot  # noqa: F821

                    def lhs(g):
                        if g < TPG:
                            return zn[:, g, :]
                        t = g % TPG
                        return txr[:, g // TPG - 1, t * CIN:(t + 1) * CIN]
                ls_t = None
                # k=1 has no collective to hide: run streamed slots first so
                # the PE starts on the first arriving group instead of
                # waiting for the resident-Ls preamble load.
                order = (list(range(RES_KT, G)) + list(range(RES_KT))
                         if k == 1 else list(range(G)))
                for idx, g in enumerate(order):
                    if g < RES_KT:
                        src = ls_res[:, g, :]
                    else:
                        gs = g - RES_KT
                        if gs % DMA_KT == 0:
                            ls_t = ls_pool.tile([128, DMA_KT, RPC], BF16,
                                                name="ls_t", tag="ls")
                            eng = nc.sync if (gs // DMA_KT) % 2 == 0 else nc.scalar
                            eng.dma_start(ls_t[:], lst_s.ap()[gs // DMA_KT])
                        src = ls_t[:, gs % DMA_KT, :]
                    for j in range(MCH):
                        nc.tensor.matmul(ys[j][:], lhs(g),
                                         src[:, j * 512:(j + 1) * 512],
                                         start=(idx == 0), stop=(idx == G - 1))

                # recurrence in fp32 (z_k = 2Y - z_{k-2}; z_1 = Y), then
                # transpose each 512-chunk to natural bf16 layout right away
                zt = zt_pool.tile([CIN, RPC], F32, name="zt", tag="zt")
                last = k == K - 1
                znat = None
                if not last:
                    znat = znat_pool.tile([128, TPG, CIN], BF16,
                                          name="znat", tag="znat")
                for j in range(MCH):
                    dst = zt[:, j * 512:(j + 1) * 512]
                    if k == 1:
                        nc.vector.tensor_copy(dst, ys[j][:])
                    else:
                        nc.vector.scalar_tensor_tensor(
                            dst, ys[j][:], 2.0,
                            zts[k - 2][:, j * 512:(j + 1) * 512],
                            mybir.AluOpType.mult, mybir.AluOpType.subtract)
                    if not last:
                        tr = tps.tile([128, 4 * CIN], F32, name="tr", tag="tr")
                        for t in range(4):
                            u = 4 * j + t
                            nc.tensor.transpose(tr[:, t * CIN:(t + 1) * CIN],
                                                zt[:, u * 128:(u + 1) * 128],
                                                ident[:])
                        nc.vector.tensor_copy(znat[:, 4 * j:4 * j + 4, :], tr[:])
                zts[k] = zt

                if not last:
                    # AllGather; land all 8 blocks in SBUF, mirror blocks
                    # 0..6 behind them (DVE), then one dynamic-offset DVE
                    # copy extracts this rank's rotated 7-block window
                    # [rank+1, rank+8). Everything after the AG runs on DVE,
                    # off the DMA rings.
                    ag_in = dram.tile([128, W_AG], BF16, name="ag_in",
                                      tag="agin")
                    ag_out = dram.tile([NCORES * 128, W_AG], BF16,
                                       name="ag_out", tag="agout",
                                       addr_space="Shared")
                    nc.gpsimd.dma_start(ag_in[:], znat[:])
                    nc.gpsimd.collective_compute(
                        "AllGather", mybir.AluOpType.bypass,
                        replica_groups=[list(range(NCORES))],
                        ins=[ag_in.opt()], outs=[ag_out.opt()])
                    tx2 = tx_pool.tile([128, 2 * NCORES - 1, W_AG], BF16,
                                       name="tx2", tag="tx2", bufs=1)
                    agv = ag_out.rearrange("(r p) w -> p r w", p=128)
                    nc.gpsimd.dma_start(tx2[:, :NCORES, :], agv[:])
                    nc.gpsimd.dma_start(tx2[:, NCORES:, :],
                                        agv[:, :NCORES - 1, :])
                    tx_rot = tx_pool.tile([128, NCORES - 1, W_AG], BF16,
                                          name="tx_rot", tag="tx")
                    nc.vector.tensor_copy(
                        tx_rot[:],
                        tx2[:, bass.ds(rot_off, NCORES - 1), :])
                    znat_prev = znat

                # projection (off the AG critical path): acc += W_k_i.T @ z.T
                for j in range(MCH):
                    pj = pps.tile([COUT, 512], F32, name="pj", tag="proj")
                    nc.tensor.matmul(pj[:], w_sb[:, k, :],
                                     zt[:, j * 512:(j + 1) * 512],
                                     start=True, stop=True)
                    nc.vector.tensor_tensor(acc[:, j * 512:(j + 1) * 512],
                                            acc[:, j * 512:(j + 1) * 512],
                                            pj[:], mybir.AluOpType.add)

            nc.sync.dma_start(out[:], acc[:])

    nc.compile()
    return nc


def _get_nc():
    if "nc" not in _NC_CACHE:
        _NC_CACHE["nc"] = _build()
    return _NC_CACHE["nc"]


def _shard(x, Ls, weight):
    in_maps = []
    xb = x.astype(NP_BF16)
    xtr = np.ascontiguousarray(x.T.astype(np.float32))
    for i in range(NCORES):
        lst_i = Ls[i * RPC:(i + 1) * RPC, :].T.astype(NP_BF16)  # [NATOT, RPC]
        # rotate angle blocks so slots 0..15 are core i's own block
        per_block = lst_i.reshape(NCORES, NATOT // NCORES, RPC)
        rot = np.concatenate([per_block[(i + sb) % NCORES]
                              for sb in range(NCORES)], axis=0)
        ls_r = np.ascontiguousarray(
            rot[:RES_KT * 128].reshape(RES_KT, 128, RPC).transpose(1, 0, 2))
        ls_s = np.ascontiguousarray(
            rot[RES_KT * 128:]
            .reshape(STREAM_KT // DMA_KT, DMA_KT, 128, RPC)
            .transpose(0, 2, 1, 3))
        w_i = np.ascontiguousarray(weight[:, i * CIN:(i + 1) * CIN, :])
        in_maps.append({"lst_s": ls_s, "lst_res": ls_r, "xb": xb, "xt": xtr,
                        "w": w_i})
    return in_maps


def run(x, Ls, weight, bias, trace=False, **kw):
    nc = _get_nc()
    in_maps = _shard(np.asarray(x), np.asarray(Ls), np.asarray(weight))
    res = run_bass_kernel_spmd(nc, in_maps, core_ids=list(range(NCORES)),
                               trace=trace, **kw)
    accs = [res.results[i]["out"] for i in range(NCORES)]
    full = np.sum(accs, axis=0, dtype=np.float32).T + np.asarray(bias)[None, :]
    return full.astype(np.float32), res


def kernel(x, Ls, weight, bias):
    out, _ = run(x, Ls, weight, bias, trace=False)
    return out


# revision 30
# speedup vs baseline: 1.0051x; 1.0051x over previous
"""Trainium2 distributed kernel for the ACSConv Chebyshev graph conv.

Math (reference): with Tx0 = tile(x, (8,1)) [16384,16],
    Tx_{k} = 2*Ls@Tx_{k-1} - Tx_{k-2}   (Tx1 = Ls@Tx0)
    out = sum_k proj(Tx_k, W_k) + bias,  proj mixes the 8 angle blocks.

Distribution (8 NeuronCores): Ls row-sharded into contiguous 2048-row
blocks (= angle blocks). Core i holds LsT_i = Ls[2048i:2048(i+1), :].T
(host pre-transposed, bf16) and computes its Tx block each iteration;
blocks are AllGather'd between iterations.

Own-block-first overlap: each core's LsT slots are host-rotated so
slots 0..15 contract against the core's OWN Tx block. Those slots are
fully local — stationary comes from the just-computed `znat` tile and
the Ls data is SBUF-resident — so the per-iteration AllGather (and the
stationary loads that follow it) overlap ~22us of DMA-free PE work at
each iteration boundary instead of stalling both PE and the Ls stream.
After the AllGather, the 8 blocks land in a doubled [15-block] SBUF
window (two parallel SWDGE DMAs), and one dynamic-offset DVE copy
extracts this rank's rotated window [rank+1, rank+8) as the stationary
tile for the 7 peer-block slot groups.

Per-core per-iteration matmul: Y.T[16, 2048] = sum_g TxTile_g.T @ LsT_g
with TxTile [128,16] stationary and LsT tiles [128, 512] moving. Slots
16..127 stream via 2 MiB DMAs (DMA-native host layout: each partition
reads DMA_KT*4KB contiguous) alternating between the Sync and Scalar
HWDGE rings. Recurrence/projection/accumulation stay in fp32; the Ls
stream and stationary Tx tiles are bf16 (sim rel-err ~6e-3, gate 2e-2).
"""

import numpy as np
import ml_dtypes

import concourse.bass as bass
import concourse.mybir as mybir
import concourse.tile as tile
from concourse import bacc
from concourse.bass_utils import run_bass_kernel_spmd
from concourse.masks import make_identity

NCORES = 8
N = 2048          # nodes
CIN = 16
COUT = 32
NANG = 8          # angles
K = 15            # Chebyshev order
NATOT = NANG * N  # 16384
RPC = NATOT // NCORES   # rows of Ls per core = 2048
G = NATOT // 128        # 128 contraction k-tiles
MCH = RPC // 512        # 4 output m-chunks of 512
TPG = N // 128          # 16 row-tiles per core block
W_AG = TPG * CIN        # 256: per-partition payload of the AG

BF16 = mybir.dt.bfloat16
F32 = mybir.dt.float32
NP_BF16 = ml_dtypes.bfloat16

RES_KT = TPG      # own-block slots 0..15 are SBUF-resident
STREAM_KT = G - RES_KT
DMA_KT = 2        # k-tiles per streaming DMA (1 MiB)
LS_BUFS = 11      # streaming prefetch depth (x DMA_KT tiles)

_NC_CACHE = {}


def _build():
    nc = bacc.Bacc("TRN2", target_bir_lowering=False, debug=False,
                   num_devices=NCORES, num_swdge_queues=2)

    lst_s = nc.dram_tensor("lst_s", [STREAM_KT // DMA_KT, 128, DMA_KT, RPC],
                           BF16, kind="ExternalInput")
    lst_res = nc.dram_tensor("lst_res", [128, RES_KT, RPC], BF16,
                             kind="ExternalInput")
    xb = nc.dram_tensor("xb", [N, CIN], BF16, kind="ExternalInput")
    xt = nc.dram_tensor("xt", [CIN, N], F32, kind="ExternalInput")
    w = nc.dram_tensor("w", [K, CIN, COUT], F32, kind="ExternalInput")
    out = nc.dram_tensor("out", [COUT, RPC], F32, kind="ExternalOutput")

    with tile.TileContext(nc) as tc:
        with (
            tc.tile_pool(name="ls", bufs=LS_BUFS) as ls_pool,
            tc.tile_pool(name="tx", bufs=2) as tx_pool,
            tc.tile_pool(name="zt", bufs=3) as zt_pool,
            tc.tile_pool(name="znat", bufs=2) as znat_pool,
            tc.tile_pool(name="small", bufs=1) as small,
            tc.tile_pool(name="yps", bufs=4, space="PSUM") as yps,
            tc.tile_pool(name="pps", bufs=2, space="PSUM") as pps,
            tc.tile_pool(name="tps", bufs=2, space="PSUM") as tps,
            tc.tile_pool(name="dram", bufs=1, space="DRAM") as dram,
        ):
            # ---- preamble ----
            ident = small.tile([CIN, CIN], F32)
            make_identity(nc, ident[:])

            xb_sb = small.tile([128, TPG, CIN], BF16)
            nc.gpsimd.dma_start(xb_sb[:], xb.ap().rearrange("(t p) c -> p t c", p=128))

            w_sb = small.tile([CIN, K, COUT], F32)
            nc.gpsimd.dma_start(w_sb[:], w.ap().rearrange("k p c -> p k c"))

            ls_res = small.tile([128, RES_KT, RPC], BF16)
            half = RES_KT // 2
            nc.sync.dma_start(ls_res[:, :half, :], lst_res[:, :half, :])
            nc.scalar.dma_start(ls_res[:, half:, :], lst_res[:, half:, :])

            # rank+1: dynamic offset of the rotated stationary window
            # (register on DVE, which performs the rotated copy)
            rot_off = nc.vector.partition_id() + 1

            # dummy AllGather, same shape as the real ones: absorbs the
            # first-collective warmup cost under iteration-1 compute
            warm_in = dram.tile([128, W_AG], BF16, name="warm_in", tag="agin")
            warm_out = dram.tile([NCORES * 128, W_AG], BF16, name="warm_out",
                                 tag="agout", addr_space="Shared")
            nc.gpsimd.dma_start(warm_in[:], xb_sb[:])
            nc.gpsimd.collective_compute(
                "AllGather", mybir.AluOpType.bypass,
                replica_groups=[list(range(NCORES))],
                ins=[warm_in.opt()], outs=[warm_out.opt()])

            # zts[0] = x.T in fp32 (Tx0 block transposed), lives in the zt pool
            xt_sb = zt_pool.tile([CIN, N], F32, name="xt_sb", tag="zt")
            nc.gpsimd.dma_start(xt_sb[:], xt[:])

            acc = small.tile([COUT, RPC], F32)
            for j in range(MCH):
                pj = pps.tile([COUT, 512], F32, name="pj", tag="proj")
                nc.tensor.matmul(pj[:], w_sb[:, 0, :], xt_sb[:, j * 512:(j + 1) * 512],
                                 start=True, stop=True)
                nc.vector.tensor_copy(acc[:, j * 512:(j + 1) * 512], pj[:])

            # ---- Chebyshev iterations k = 1..14 ----
            zts = {0: xt_sb}
            znat_prev = None
            for k in range(1, K):
                # Y.T = (Ls_i @ Tx_{k-1}).T accumulated over the 128 slots
                ys = [yps.tile([CIN, 512], F32, name=f"y{j}", tag="y")
                      for j in range(MCH)]
                if k == 1:
                    def lhs(g):
                        return xb_sb[:, g % TPG, :]
                else:
                    zn, txr = znat_prev, tx_rot  # noqa: F821

                    def lhs(g):
                        if g < TPG:
                            return zn[:, g, :]
                        t = g % TPG
                        return txr[:, g // TPG - 1, t * CIN:(t + 1) * CIN]
                ls_t = None
                for g in range(G):
                    if g < RES_KT:
                        src = ls_res[:, g, :]
                    else:
                        gs = g - RES_KT
                        if gs % DMA_KT == 0:
                            ls_t = ls_pool.tile([128, DMA_KT, RPC], BF16,
                                                name="ls_t", tag="ls")
                            eng = nc.sync if (gs // DMA_KT) % 2 == 0 else nc.scalar
                            eng.dma_start(ls_t[:], lst_s.ap()[gs // DMA_KT])
                        src = ls_t[:, gs % DMA_KT, :]
                    for j in range(MCH):
                        nc.tensor.matmul(ys[j][:], lhs(g),
                                         src[:, j * 512:(j + 1) * 512],
                                         start=(g == 0), stop=(g == G - 1))

                # recurrence in fp32 (z_k = 2Y - z_{k-2}; z_1 = Y), then
                # transpose each 512-chunk to natural bf16 layout right away
                zt = zt_pool.tile([CIN, RPC], F32, name="zt", tag="zt")
                last = k == K - 1
                znat = None
                if not last:
                    znat = znat_pool.tile([128, TPG, CIN], BF16,
                                          name="znat", tag="znat")
                for j in range(MCH):
                    dst = zt[:, j * 512:(j + 1) * 512]
                    if k == 1:
                        nc.vector.tensor_copy(dst, ys[j][:])
                    else:
                        nc.vector.scalar_tensor_tensor(
                            dst, ys[j][:], 2.0,
                            zts[k - 2][:, j * 512:(j + 1) * 512],
                            mybir.AluOpType.mult, mybir.AluOpType.subtract)
                    if not last:
                        tr = tps.tile([128, 4 * CIN], F32, name="tr", tag="tr")
                        for t in range(4):
                            u = 4 * j + t
                            nc.tensor.transpose(tr[:, t * CIN:(t + 1) * CIN],
                                                zt[:, u * 128:(u + 1) * 128],
                                                ident[:])
                        nc.vector.tensor_copy(znat[:, 4 * j:4 * j + 4, :], tr[:])
                zts[k] = zt

                if not last:
                    # AllGather; land all 8 blocks in SBUF, mirror blocks
                    # 0..6 behind them (DVE), then one dynamic-offset DVE
                    # copy extracts this rank's rotated 7-block window
                    # [rank+1, rank+8). Everything after the AG runs on DVE,
                    # off the DMA rings.
                    ag_in = dram.tile([128, W_AG], BF16, name="ag_in",
                                      tag="agin")
                    ag_out = dram.tile([NCORES * 128, W_AG], BF16,
                                       name="ag_out", tag="agout",
                                       addr_space="Shared")
                    nc.gpsimd.dma_start(ag_in[:], znat[:])
                    nc.gpsimd.collective_compute(
                        "AllGather", mybir.AluOpType.bypass,
                        replica_groups=[list(range(NCORES))],
                        ins=[ag_in.opt()], outs=[ag_out.opt()])
                    tx2 = tx_pool.tile([128, 2 * NCORES - 1, W_AG], BF16,
                                       name="tx2", tag="tx2", bufs=1)
                    agv = ag_out.rearrange("(r p) w -> p r w", p=128)
                    nc.gpsimd.dma_start(tx2[:, :NCORES, :], agv[:])
                    nc.gpsimd.dma_start(tx2[:, NCORES:, :],
                                        agv[:, :NCORES - 1, :])
                    tx_rot = tx_pool.tile([128, NCORES - 1, W_AG], BF16,
                                          name="tx_rot", tag="tx")
                    nc.vector.tensor_copy(
                        tx_rot[:],
                        tx2[:, bass.ds(rot_off, NCORES - 1), :])
                    znat_prev = znat

                # projection (off the AG critical path): acc += W_k_i.T @ z.T
                for j in range(MCH):
                    pj = pps.tile([COUT, 512], F32, name="pj", tag="proj")
                    nc.tensor.matmul(pj[:], w_sb[:, k, :],
                                     zt[:, j * 512:(j + 1) * 512],
                                     start=True, stop=True)
                    nc.vector.tensor_tensor(acc[:, j * 512:(j + 1) * 512],
                                            acc[:, j * 512:(j + 1) * 512],
                                            pj[:], mybir.AluOpType.add)

            nc.sync.dma_start(out[:], acc[:])

    nc.compile()
    return nc


def _get_nc():
    if "nc" not in _NC_CACHE:
        _NC_CACHE["nc"] = _build()
    return _NC_CACHE["nc"]


def _shard(x, Ls, weight):
    in_maps = []
    xb = x.astype(NP_BF16)
    xtr = np.ascontiguousarray(x.T.astype(np.float32))
    for i in range(NCORES):
        lst_i = Ls[i * RPC:(i + 1) * RPC, :].T.astype(NP_BF16)  # [NATOT, RPC]
        # rotate angle blocks so slots 0..15 are core i's own block
        per_block = lst_i.reshape(NCORES, NATOT // NCORES, RPC)
        rot = np.concatenate([per_block[(i + sb) % NCORES]
                              for sb in range(NCORES)], axis=0)
        ls_r = np.ascontiguousarray(
            rot[:RES_KT * 128].reshape(RES_KT, 128, RPC).transpose(1, 0, 2))
        ls_s = np.ascontiguousarray(
            rot[RES_KT * 128:]
            .reshape(STREAM_KT // DMA_KT, DMA_KT, 128, RPC)
            .transpose(0, 2, 1, 3))
        w_i = np.ascontiguousarray(weight[:, i * CIN:(i + 1) * CIN, :])
        in_maps.append({"lst_s": ls_s, "lst_res": ls_r, "xb": xb, "xt": xtr,
                        "w": w_i})
    return in_maps


def run(x, Ls, weight, bias, trace=False, **kw):
    nc = _get_nc()
    in_maps = _shard(np.asarray(x), np.asarray(Ls), np.asarray(weight))
    res = run_bass_kernel_spmd(nc, in_maps, core_ids=list(range(NCORES)),
                               trace=trace, **kw)
    accs = [res.results[i]["out"] for i in range(NCORES)]
    full = np.sum(accs, axis=0, dtype=np.float32).T + np.asarray(bias)[None, :]
    return full.astype(np.float32), res


def kernel(x, Ls, weight, bias):
    out, _ = run(x, Ls, weight, bias, trace=False)
    return out


# revision 32
# speedup vs baseline: 1.0287x; 1.0234x over previous
"""Trainium2 distributed kernel for the ACSConv Chebyshev graph conv.

Math (reference): with Tx0 = tile(x, (8,1)) [16384,16],
    Tx_{k} = 2*Ls@Tx_{k-1} - Tx_{k-2}   (Tx1 = Ls@Tx0)
    out = sum_k proj(Tx_k, W_k) + bias,  proj mixes the 8 angle blocks.

Distribution (8 NeuronCores): Ls row-sharded into contiguous 2048-row
blocks (= angle blocks). Core i holds LsT_i = Ls[2048i:2048(i+1), :].T
(host pre-transposed, bf16) and computes its Tx block each iteration;
blocks are AllGather'd between iterations.

Own-block-first overlap: each core's LsT slots are host-rotated so
slots 0..15 contract against the core's OWN Tx block. Those slots are
fully local — stationary comes from the just-computed `znat` tile and
the Ls data is SBUF-resident — so the per-iteration AllGather (and the
stationary loads that follow it) overlap ~22us of DMA-free PE work at
each iteration boundary instead of stalling both PE and the Ls stream.
After the AllGather, the 8 blocks land in a doubled [15-block] SBUF
window (two parallel SWDGE DMAs), and one dynamic-offset DVE copy
extracts this rank's rotated window [rank+1, rank+8) as the stationary
tile for the 7 peer-block slot groups.

Per-core per-iteration matmul: Y.T[16, 2048] = sum_g TxTile_g.T @ LsT_g
with TxTile [128,16] stationary and LsT tiles [128, 512] moving. Slots
16..127 stream via 2 MiB DMAs (DMA-native host layout: each partition
reads DMA_KT*4KB contiguous) alternating between the Sync and Scalar
HWDGE rings. Recurrence/projection/accumulation stay in fp32; the Ls
stream and stationary Tx tiles are bf16 (sim rel-err ~6e-3, gate 2e-2).
"""

import numpy as np
import ml_dtypes

import concourse.bass as bass
import concourse.mybir as mybir
import concourse.tile as tile
from concourse import bacc
from concourse.bass_utils import run_bass_kernel_spmd
from concourse.masks import make_identity

NCORES = 8
N = 2048          # nodes
CIN = 16
COUT = 32
NANG = 8          # angles
K = 15            # Chebyshev order
NATOT = NANG * N  # 16384
RPC = NATOT // NCORES   # rows of Ls per core = 2048
G = NATOT // 128        # 128 contraction k-tiles
MCH = RPC // 512        # 4 output m-chunks of 512
TPG = N // 128          # 16 row-tiles per core block
W_AG = TPG * CIN        # 256: per-partition payload of the AG

BF16 = mybir.dt.bfloat16
F32 = mybir.dt.float32
NP_BF16 = ml_dtypes.bfloat16

RES_KT = TPG      # own-block slots 0..15 are SBUF-resident
STREAM_KT = G - RES_KT
DMA_KT = 2        # k-tiles per streaming DMA (1 MiB)
LS_BUFS = 11      # streaming prefetch depth (x DMA_KT tiles)

_NC_CACHE = {}


def _build():
    nc = bacc.Bacc("TRN2", target_bir_lowering=False, debug=False,
                   num_devices=NCORES, num_swdge_queues=2)

    lst_s = nc.dram_tensor("lst_s", [STREAM_KT // DMA_KT, 128, DMA_KT, RPC],
                           BF16, kind="ExternalInput")
    lst_res = nc.dram_tensor("lst_res", [128, RES_KT, RPC], BF16,
                             kind="ExternalInput")
    xb = nc.dram_tensor("xb", [N, CIN], BF16, kind="ExternalInput")
    xt = nc.dram_tensor("xt", [CIN, N], F32, kind="ExternalInput")
    w = nc.dram_tensor("w", [K, CIN, COUT], F32, kind="ExternalInput")
    out = nc.dram_tensor("out", [COUT, RPC], F32, kind="ExternalOutput")

    with tile.TileContext(nc) as tc:
        with (
            tc.tile_pool(name="ls", bufs=LS_BUFS) as ls_pool,
            tc.tile_pool(name="tx", bufs=2) as tx_pool,
            tc.tile_pool(name="zt", bufs=3) as zt_pool,
            tc.tile_pool(name="znat", bufs=2) as znat_pool,
            tc.tile_pool(name="small", bufs=1) as small,
            tc.tile_pool(name="yps", bufs=4, space="PSUM") as yps,
            tc.tile_pool(name="pps", bufs=2, space="PSUM") as pps,
            tc.tile_pool(name="tps", bufs=2, space="PSUM") as tps,
            tc.tile_pool(name="dram", bufs=1, space="DRAM") as dram,
        ):
            # ---- preamble ----
            ident = small.tile([CIN, CIN], F32)
            make_identity(nc, ident[:])

            xb_sb = small.tile([128, TPG, CIN], BF16)
            nc.gpsimd.dma_start(xb_sb[:], xb.ap().rearrange("(t p) c -> p t c", p=128))

            w_sb = small.tile([CIN, K, COUT], F32)
            nc.gpsimd.dma_start(w_sb[:], w.ap().rearrange("k p c -> p k c"))

            ls_res = small.tile([128, RES_KT, RPC], BF16)
            half = RES_KT // 2
            nc.sync.dma_start(ls_res[:, :half, :], lst_res[:, :half, :])
            nc.scalar.dma_start(ls_res[:, half:, :], lst_res[:, half:, :])

            # rank+1: dynamic offset of the rotated stationary window
            # (register on DVE, which performs the rotated copy)
            rot_off = nc.vector.partition_id() + 1

            # dummy AllGather, same shape as the real ones: absorbs the
            # first-collective warmup cost under iteration-1 compute
            warm_in = dram.tile([128, W_AG], BF16, name="warm_in", tag="agin")
            warm_out = dram.tile([NCORES * 128, W_AG], BF16, name="warm_out",
                                 tag="agout", addr_space="Shared")
            nc.gpsimd.dma_start(warm_in[:], xb_sb[:])
            nc.gpsimd.collective_compute(
                "AllGather", mybir.AluOpType.bypass,
                replica_groups=[list(range(NCORES))],
                ins=[warm_in.opt()], outs=[warm_out.opt()])

            # zts[0] = x.T in fp32 (Tx0 block transposed), lives in the zt pool
            xt_sb = zt_pool.tile([CIN, N], F32, name="xt_sb", tag="zt")
            nc.gpsimd.dma_start(xt_sb[:], xt[:])

            acc = small.tile([COUT, RPC], F32)
            for j in range(MCH):
                pj = pps.tile([COUT, 512], F32, name="pj", tag="proj")
                nc.tensor.matmul(pj[:], w_sb[:, 0, :], xt_sb[:, j * 512:(j + 1) * 512],
                                 start=True, stop=True)
                nc.vector.tensor_copy(acc[:, j * 512:(j + 1) * 512], pj[:])

            # ---- Chebyshev iterations k = 1..14 ----
            zts = {0: xt_sb}
            znat_prev = None
            for k in range(1, K):
                # Y.T = (Ls_i @ Tx_{k-1}).T accumulated over the 128 slots
                ys = [yps.tile([CIN, 512], F32, name=f"y{j}", tag="y")
                      for j in range(MCH)]
                if k == 1:
                    def lhs(g):
                        return xb_sb[:, g % TPG, :]
                else:
                    zn, txr = znat_prev, tx_rot  # noqa: F821

                    def lhs(g):
                        if g < TPG:
                            return zn[:, g, :]
                        t = g % TPG
                        return txr[:, g // TPG - 1, t * CIN:(t + 1) * CIN]
                ls_t = None
                for g in range(G):
                    if g < RES_KT:
                        src = ls_res[:, g, :]
                    else:
                        gs = g - RES_KT
                        if gs % DMA_KT == 0:
                            ls_t = ls_pool.tile([128, DMA_KT, RPC], BF16,
                                                name="ls_t", tag="ls")
                            eng = nc.sync if (gs // DMA_KT) % 2 == 0 else nc.scalar
                            eng.dma_start(ls_t[:], lst_s.ap()[gs // DMA_KT])
                        src = ls_t[:, gs % DMA_KT, :]
                    for j in range(MCH):
                        nc.tensor.matmul(ys[j][:], lhs(g),
                                         src[:, j * 512:(j + 1) * 512],
                                         start=(g == 0), stop=(g == G - 1))

                # recurrence in fp32 (z_k = 2Y - z_{k-2}; z_1 = Y), then
                # transpose each 512-chunk to natural bf16 layout right away
                zt = zt_pool.tile([CIN, RPC], F32, name="zt", tag="zt")
                last = k == K - 1
                znat = None
                if not last:
                    znat = znat_pool.tile([128, TPG, CIN], BF16,
                                          name="znat", tag="znat")
                for j in range(MCH):
                    dst = zt[:, j * 512:(j + 1) * 512]
                    if k == 1:
                        nc.vector.tensor_copy(dst, ys[j][:])
                    else:
                        nc.vector.scalar_tensor_tensor(
                            dst, ys[j][:], 2.0,
                            zts[k - 2][:, j * 512:(j + 1) * 512],
                            mybir.AluOpType.mult, mybir.AluOpType.subtract)
                    if not last:
                        tr = tps.tile([128, 4 * CIN], F32, name="tr", tag="tr")
                        for t in range(4):
                            u = 4 * j + t
                            nc.tensor.transpose(tr[:, t * CIN:(t + 1) * CIN],
                                                zt[:, u * 128:(u + 1) * 128],
                                                ident[:])
                        nc.vector.tensor_copy(znat[:, 4 * j:4 * j + 4, :], tr[:])
                zts[k] = zt

                if not last:
                    # AllGather; land all 8 blocks in SBUF, mirror blocks
                    # 0..6 behind them (DVE), then one dynamic-offset DVE
                    # copy extracts this rank's rotated 7-block window
                    # [rank+1, rank+8). Everything after the AG runs on DVE,
                    # off the DMA rings.
                    ag_in = dram.tile([128, W_AG], BF16, name="ag_in",
                                      tag="agin")
                    ag_out = dram.tile([NCORES * 128, W_AG], BF16,
                                       name="ag_out", tag="agout",
                                       addr_space="Shared")
                    nc.gpsimd.dma_start(ag_in[:], znat[:])
                    nc.gpsimd.collective_compute(
                        "AllGather", mybir.AluOpType.bypass,
                        replica_groups=[list(range(NCORES))],
                        ins=[ag_in.opt()], outs=[ag_out.opt()])
                    tx2 = tx_pool.tile([128, 2 * NCORES - 1, W_AG], BF16,
                                       name="tx2", tag="tx2", bufs=1)
                    agv = ag_out.rearrange("(r p) w -> p r w", p=128)
                    nc.gpsimd.dma_start(tx2[:, :NCORES, :], agv[:])
                    nc.gpsimd.dma_start(tx2[:, NCORES:, :],
                                        agv[:, :NCORES - 1, :])
                    tx_rot = tx_pool.tile([128, NCORES - 1, W_AG], BF16,
                                          name="tx_rot", tag="tx")
                    nc.vector.tensor_copy(
                        tx_rot[:],
                        tx2[:, bass.ds(rot_off, NCORES - 1), :])
                    znat_prev = znat

                # projection (off the AG critical path): acc += W_k_i.T @ z.T
                for j in range(MCH):
                    pj = pps.tile([COUT, 512], F32, name="pj", tag="proj")
                    nc.tensor.matmul(pj[:], w_sb[:, k, :],
                                     zt[:, j * 512:(j + 1) * 512],
                                     start=True, stop=True)
                    nc.vector.tensor_tensor(acc[:, j * 512:(j + 1) * 512],
                                            acc[:, j * 512:(j + 1) * 512],
                                            pj[:], mybir.AluOpType.add)

            nc.sync.dma_start(out[:], acc[:])

    nc.compile()
    return nc


def _get_nc():
    if "nc" not in _NC_CACHE:
        _NC_CACHE["nc"] = _build()
    return _NC_CACHE["nc"]


def _shard(x, Ls, weight):
    in_maps = []
    xb = x.astype(NP_BF16)
    xtr = np.ascontiguousarray(x.T.astype(np.float32))
    for i in range(NCORES):
        lst_i = Ls[i * RPC:(i + 1) * RPC, :].T.astype(NP_BF16)  # [NATOT, RPC]
        # rotate angle blocks so slots 0..15 are core i's own block
        per_block = lst_i.reshape(NCORES, NATOT // NCORES, RPC)
        rot = np.concatenate([per_block[(i + sb) % NCORES]
                              for sb in range(NCORES)], axis=0)
        ls_r = np.ascontiguousarray(
            rot[:RES_KT * 128].reshape(RES_KT, 128, RPC).transpose(1, 0, 2))
        ls_s = np.ascontiguousarray(
            rot[RES_KT * 128:]
            .reshape(STREAM_KT // DMA_KT, DMA_KT, 128, RPC)
            .transpose(0, 2, 1, 3))
        w_i = np.ascontiguousarray(weight[:, i * CIN:(i + 1) * CIN, :])
        in_maps.append({"lst_s": ls_s, "lst_res": ls_r, "xb": xb, "xt": xtr,
                        "w": w_i})
    return in_maps


def run(x, Ls, weight, bias, trace=False, **kw):
    nc = _get_nc()
    in_maps = _shard(np.asarray(x), np.asarray(Ls), np.asarray(weight))
    res = run_bass_kernel_spmd(nc, in_maps, core_ids=list(range(NCORES)),
                               trace=trace, **kw)
    accs = [res.results[i]["out"] for i in range(NCORES)]
    full = np.sum(accs, axis=0, dtype=np.float32).T + np.asarray(bias)[None, :]
    return full.astype(np.float32), res


def kernel(x, Ls, weight, bias):
    out, _ = run(x, Ls, weight, bias, trace=False)
    return out
